# revision 14
# baseline (speedup 1.0000x reference)
"""Multi-head causal attention (B=4, T=2048, C=1024, H=16) on 8 TRN2 NeuronCores.

Sharding: core c handles batch b = c//2 and head-group g = c%2 (8 heads,
512 output channels). Host passes per-core transposed bf16 operands (x[b].T
and W[rows].T) so every on-device matmul streams natural-layout tiles:

  q.T[d,i] = sum_c WqT[c,d] * xT[c,i]   -> matmul(lhsT=WqT tile, rhs=xT tile)
  v[t,e]   = sum_c xT[c,t]  * WvT[c,e]  -> matmul(lhsT=xT tile,  rhs=WvT tile)
  S.T[j,i] = sum_d kT[d,j]  * qT[d,i]   -> matmul(lhsT=kT tile,  rhs=qT tile)
  O.T[e,i] = sum_j v[j,e]   * P[j,i]    -> matmul(lhsT=v tile,   rhs=exp tile)

V carries an appended ones column per head so row 64 of the O.T accumulator
is the softmax denominator; the numerator/denominator division happens on the
host after gather. Causal masking is a 0/1 multiply on the exp tile
(exp(-inf) == 0) over only the 128x128 triangle block where the diagonal
actually crosses; query columns that are fully below the diagonal are
column-skipped in the S matmul, the exp, and the O matmul. No max-subtraction
pass: scores*scale ~ N(0,1), so exp stays comfortably inside f32/bf16 range.

The attention phase is ScalarE(exp)-bound with ~25% PE idle, so the dt>=1
q/k projection matmul groups are drip-fed one per 5 attention steps to fill
the PE's exp-wait bubbles; head-pair p's attention emits the projections for
pair p+1, which complete before they are read. PSUM plan (8 banks): S
supertiles 2x2 (double-buffered), O accumulators 2x1 (single accumulator per
head; consecutive O matmuls alternate the two heads' banks so same-bank
accumulate turnaround is dodged), projection pool 2x1. Output blocks DMA
straight from PSUM to DRAM.
"""
import numpy as np

import bass_rust
import concourse.bass as bass
import concourse.mybir as mybir
import concourse.tile as tile
from concourse.bass_utils import run_bass_kernel_spmd

P = 128
HS = 64  # head size


def _split_sync_waits(nc, max_waits=1):
    # This walrus build's setupSyncWait admits a single sync-wait slot per
    # instruction, but Tile can emit several (cross-proc deps on one inst).
    # Peel extra waits onto preceding same-engine NOPs (pure wait carriers;
    # a Drain would flush the PE pipe).
    all_bbs = [b for fn in nc.m.functions for b in fn.blocks]
    for bb in all_bbs:
        insts = bb.instructions
        i = 0
        while i < len(insts):
            inst = insts[i]
            si = inst.sync_info
            ow = list(si.on_wait) if si and si.on_wait else []
            if len(ow) > max_waits:
                keep = ow[-max_waits:]
                rest = ow[:-max_waits]
                eng = nc.engines[inst.engine]
                new_insts = []
                while rest:
                    chunk, rest = rest[:max_waits], rest[max_waits:]
                    d = eng.nop()
                    d.ins.sync_info = bass_rust.SyncInfo(on_wait=chunk, on_update=[])
                    new_insts.append(d.ins)
                for bb2 in all_bbs:
                    ilist = bb2.instructions
                    changed = False
                    for ni in new_insts:
                        if ni in ilist:
                            ilist.remove(ni)
                            changed = True
                    if changed:
                        bb2.instructions = ilist
                si.on_wait = keep
                bb.instructions = insts[:i] + new_insts + insts[i:]
                insts = bb.instructions
                i += len(new_insts)
            i += 1
    return nc


def _make_tri():
    # tri[p, 0, i] = 1 if p <= i else 0 (the diagonal 128x128 causal triangle)
    import ml_dtypes

    p = np.arange(P)[:, None]
    i = np.arange(P)[None, :]
    return (p <= i).astype(ml_dtypes.bfloat16).reshape(P, 1, P)


def build_nc(T=2048, C=1024, D=512, FB=512, interleave=True):
    """One-core SPMD program: xT (C,T), wqT/wkT/wvT (C,D) bf16
    -> out_nd (H, HS+1, T) f32 numerators + denominator rows."""
    f32 = mybir.dt.float32
    bf16 = mybir.dt.bfloat16
    CK = C // P  # contraction subtiles
    DT = D // P  # q/k d-tiles
    TT = T // P  # t-tiles (v rows / key tiles)
    TB = T // FB  # query blocks
    JB = FB // P  # key tiles per query block
    H = D // HS  # local heads
    HPD = P // HS  # heads per d-tile (2)
    scale = float(HS) ** -0.5

    nc = bass.Bass()
    xT = nc.declare_dram_parameter("xT", [C, T], bf16, isOutput=False)
    wqT = nc.declare_dram_parameter("wqT", [C, D], bf16, isOutput=False)
    wkT = nc.declare_dram_parameter("wkT", [C, D], bf16, isOutput=False)
    wvT = nc.declare_dram_parameter("wvT", [C, D], bf16, isOutput=False)
    out_nd = nc.declare_dram_parameter("out_nd", [H, HS + 1, T], bf16, isOutput=True)
    tri = nc.inline_tensor(_make_tri(), name="tri_mask")

    xT_r = xT.rearrange("(ck p) t -> p ck t", p=P)
    wT_r = {w.name: w.rearrange("(ck p) d -> p ck d", p=P) for w in (wqT, wkT, wvT)}

    from contextlib import ExitStack

    with tile.TileContext(nc) as tc:
        with (
            tc.tile_pool(name="persist", bufs=1) as persist,
            tc.tile_pool(name="work", bufs=6) as work,
        ):
            # ---- load bf16 inputs (per-chunk tiles so deps stay fine-grained;
            # x/wq/wk chunks interleaved so q/k proj matmuls start as soon as
            # chunk 0 lands; wv after (v tiles are consumed later); tri last)
            # x chunks stream alone on the Sync HWDGE queue; the weights go on
            # the Scalar engine's HWDGE queue in parallel (ACT is idle during
            # the load), two chunks per descriptor to bound issue time
            x_bf = []
            w_bf = {w.name: [] for w in (wqT, wkT, wvT)}
            for ck in range(CK):
                t_ = persist.tile([P, T], bf16, name=f"x_bf{ck}", tag=f"x_bf{ck}")
                nc.sync.dma_start(t_[:], xT_r[:, ck, :])
                x_bf.append(t_)
            for w in (wqT, wkT, wvT):
                for ck2 in range(CK // 2):
                    t_ = persist.tile(
                        [P, 2, D], bf16, name=f"{w.name}_bf{ck2}", tag=f"{w.name}_bf{ck2}"
                    )
                    nc.scalar.dma_start(t_[:], wT_r[w.name][:, 2 * ck2 : 2 * ck2 + 2, :])
                    w_bf[w.name].append(t_)

            def w_sl(w, ck, cols):
                return w_bf[w.name][ck // 2][:, ck % 2, cols]

            tri_sb = persist.tile([P, 1, P], bf16, name="tri_sb", tag="tri_sb")
            nc.scalar.dma_start(tri_sb[:], tri[:])

            qT = [persist.tile([P, T], bf16, name=f"qT{dt}", tag=f"qT{dt}") for dt in range(DT)]
            kT = [persist.tile([P, T], bf16, name=f"kT{dt}", tag=f"kT{dt}") for dt in range(DT)]
            # v with an appended ones column per head: [P, TT, H*(HS+1)]
            v_sb = persist.tile([P, TT, H * (HS + 1)], bf16, tag="v_sb")

            def emit_qk_group(pool, w, dt, bi):
                # one projection group: CK accumulating matmuls + copy-out cast
                dst = qT[dt] if w is wqT else kT[dt]
                ps = pool.tile([P, FB], f32, name="ps_proj", tag="ps_proj")
                for ck in range(CK):
                    nc.tensor.matmul(
                        ps[:],
                        lhsT=w_sl(w, ck, slice(dt * P, (dt + 1) * P)),
                        rhs=x_bf[ck][:, bi * FB : (bi + 1) * FB],
                        start=(ck == 0),
                        stop=(ck == CK - 1),
                    )
                nc.vector.tensor_copy(dst[:, bi * FB : (bi + 1) * FB], ps[:])

            def emit_v_group(pool, tt):
                ps = pool.tile([P, D], f32, name="ps_proj", tag="ps_proj")
                for ck in range(CK):
                    nc.tensor.matmul(
                        ps[:],
                        lhsT=x_bf[ck][:, tt * P : (tt + 1) * P],
                        rhs=w_sl(wvT, ck, slice(0, D)),
                        start=(ck == 0),
                        stop=(ck == CK - 1),
                    )
                v3 = v_sb[:, tt, :].rearrange("p (h e) -> p h e", e=HS + 1)
                nc.vector.tensor_copy(v3[:, :, 0:HS], ps[:].rearrange("p (h e) -> p h e", e=HS))
                nc.vector.memset(v3[:, :, HS : HS + 1], 1.0)

            # ---- minimal upfront phase (wide PSUM ring so the chunk-paced
            # start keeps several accumulation groups in flight): q/k for
            # head-pair 0 / query-block 0, plus the first JB v tiles ----
            up_stack = ExitStack()
            ps_up = up_stack.enter_context(
                tc.tile_pool(name="ps_up", bufs=6, space="PSUM")
            )
            emit_qk_group(ps_up, wqT, 0, 0)
            emit_qk_group(ps_up, wkT, 0, 0)
            for tt in range(JB):
                emit_v_group(ps_up, tt)
            up_stack.close()

            # everything else drips into attention's PE bubbles as fillers,
            # ordered by its consumption deadline
            def filler_list():
                fills = []
                fills += [(emit_qk_group, wqT, 0, 1), (emit_qk_group, wkT, 0, 1)]
                fills += [(emit_v_group, tt) for tt in (4, 5)]
                fills += [(emit_qk_group, wqT, 0, 2), (emit_qk_group, wkT, 0, 2)]
                fills += [(emit_v_group, tt) for tt in (6, 7, 8, 9)]
                fills += [(emit_qk_group, wqT, 0, 3), (emit_qk_group, wkT, 0, 3)]
                fills += [(emit_v_group, tt) for tt in range(10, TT)]
                for dt in range(1, DT):
                    for w in (wqT, wkT):
                        for bi in range(TB):
                            fills.append((emit_qk_group, w, dt, bi))
                return fills

            fillers = iter(filler_list())

            def inject(pool, s):
                # deadline-driven: 2-of-3 steps early (head-pair-0 feeders +
                # dt1 due by step 40), then every 5th (dt2 by 80, dt3 by 120)
                if (s < 39 and s % 3 != 2) or (39 <= s < 120 and s % 5 == 0):
                    f = next(fillers, None)
                    if f is not None:
                        f[0](pool, *f[1:])

            if not interleave:
                for f in fillers:
                    f[0](ps_up, *f[1:])
                fillers = iter(())

            attn_stack = ExitStack()
            ps_s_pool = attn_stack.enter_context(
                tc.tile_pool(name="ps_s", bufs=2, space="PSUM")
            )
            ps_o_pool = attn_stack.enter_context(
                tc.tile_pool(name="ps_o", bufs=2, space="PSUM")
            )
            ps_proj = attn_stack.enter_context(
                tc.tile_pool(name="ps_proj", bufs=2, space="PSUM")
            )

            # ---- causal attention: one flat software-pipelined stream across
            # all (head-pair, query-block) blocks; S/exp of step s+1 issue
            # before O of step s, including across block boundaries, so the
            # next block's S-matmuls stream while the last O of the previous
            # block waits for its exp ----
            steps = [
                (hp, bi, jt)
                for hp in range(H // HPD)
                for bi in range(TB)
                for jt in range((bi + 1) * JB)
            ]
            ns = len(steps)
            ps_o_map = {}
            ex_map = {}
            for s in range(ns + 1):
                if s < ns:
                    hp, bi, jt = steps[s]
                    dt = hp
                    if jt == 0:
                        ps_o_map[(hp, bi)] = [
                            ps_o_pool.tile([HS + 1, FB], f32, name=f"ps_o{par}", tag="ps_o")
                            for par in range(HPD)
                        ]
                    l = jt - bi * JB  # >=0 on diagonal tiles
                    c0 = max(l, 0) * P  # fully-masked query-column prefix
                    # both heads' S tiles land in one 2-bank super-tile so a
                    # single EXP covers them (amortizes the ~200ns overhead)
                    sup = ps_s_pool.tile([P, HPD, FB], f32, name="ps_s", tag="ps_s")
                    for par in range(HPD):
                        po = HS * par
                        nc.tensor.matmul(
                            sup[:, par, c0:FB],
                            lhsT=kT[dt][po : po + HS, jt * P : (jt + 1) * P],
                            rhs=qT[dt][po : po + HS, bi * FB + c0 : (bi + 1) * FB],
                            start=True,
                            stop=True,
                        )
                    ex = work.tile([P, HPD, FB], bf16, name="exp", tag="exp")
                    nc.scalar.activation(
                        ex[:, :, c0:FB],
                        sup[:, :, c0:FB],
                        mybir.ActivationFunctionType.Exp,
                        scale=scale,
                    )
                    if l >= 0:
                        # only the 128-wide block where the diagonal crosses
                        # needs masking; columns beyond are fully visible,
                        # columns before were skipped
                        nc.vector.tensor_mul(
                            ex[:, :, c0 : c0 + P],
                            ex[:, :, c0 : c0 + P],
                            tri_sb[:].to_broadcast((P, HPD, P)),
                        )
                    ex_map[s] = (ex, c0)
                if s > 0:
                    hp, bi, jt = steps[s - 1]
                    nj = (bi + 1) * JB
                    ex, c0 = ex_map.pop(s - 1)
                    ps_o = ps_o_map[(hp, bi)]
                    for par in range(HPD):
                        h = HPD * hp + par
                        nc.tensor.matmul(
                            ps_o[par][:, c0:FB],
                            lhsT=v_sb[:, jt, (HS + 1) * h : (HS + 1) * (h + 1)],
                            rhs=ex[:, par, c0:FB],
                            start=(jt == 0),
                            stop=(jt == nj - 1),
                        )
                    if jt == nj - 1:
                        for par in range(HPD):
                            h = HPD * hp + par
                            nd = work.tile([HS + 1, FB], bf16, name="nd", tag="nd")
                            nc.vector.tensor_copy(nd[:], ps_o[par][:])
                            nc.sync.dma_start(
                                out_nd[h, :, bi * FB : (bi + 1) * FB], nd[:]
                            )
                        del ps_o_map[(hp, bi)]
                inject(ps_proj, s)

            for f in fillers:
                f[0](ps_proj, *f[1:])
            attn_stack.close()

    _split_sync_waits(nc)
    return nc


_NC_CACHE = {}


def _get_nc(key=(2048, 1024, 512, 512)):
    if key not in _NC_CACHE:
        _NC_CACHE[key] = build_nc(*key)
    return _NC_CACHE[key]


def run(x, Wq, Wk, Wv, trace=False, **spmd_kwargs):
    import ml_dtypes

    bf = ml_dtypes.bfloat16
    B, T, C = x.shape
    n_cores = 8
    gpb = 2  # head-groups per batch
    D = C // gpb
    H = D // HS

    nc = _get_nc((T, C, D, 512))

    in_maps = []
    for c in range(n_cores):
        b, g = c // gpb, c % gpb
        rows = slice(g * D, (g + 1) * D)
        in_maps.append(
            {
                "xT": np.ascontiguousarray(np.asarray(x)[b].T).astype(bf),
                "wqT": np.ascontiguousarray(np.asarray(Wq)[rows].T).astype(bf),
                "wkT": np.ascontiguousarray(np.asarray(Wk)[rows].T).astype(bf),
                "wvT": np.ascontiguousarray(np.asarray(Wv)[rows].T).astype(bf),
            }
        )

    res = run_bass_kernel_spmd(
        nc, in_maps, core_ids=list(range(n_cores)), trace=trace, **spmd_kwargs
    )

    out = np.empty((B, T, C), np.float32)
    for c in range(n_cores):
        b, g = c // gpb, c % gpb
        r = np.asarray(res.results[c]["out_nd"], np.float32)  # (H, HS+1, T)
        o = r[:, :HS, :] / r[:, HS : HS + 1, :]  # (H, HS, T)
        out[b, :, g * D : (g + 1) * D] = o.transpose(2, 0, 1).reshape(T, D)
    return out, res


def kernel(x, Wq, Wk, Wv, **_):
    out, _res = run(x, Wq, Wk, Wv, trace=False)
    return out


# revision 17
# speedup vs baseline: 1.0159x; 1.0159x over previous
"""Multi-head causal attention (B=4, T=2048, C=1024, H=16) on 8 TRN2 NeuronCores.

Sharding: core c handles batch b = c//2 and head-group g = c%2 (8 heads,
512 output channels). Host passes per-core transposed bf16 operands (x[b].T
and W[rows].T) so every on-device matmul streams natural-layout tiles:

  q.T[d,i] = sum_c WqT[c,d] * xT[c,i]   -> matmul(lhsT=WqT tile, rhs=xT tile)
  v[t,e]   = sum_c xT[c,t]  * WvT[c,e]  -> matmul(lhsT=xT tile,  rhs=WvT tile)
  S.T[j,i] = sum_d kT[d,j]  * qT[d,i]   -> matmul(lhsT=kT tile,  rhs=qT tile)
  O.T[e,i] = sum_j v[j,e]   * P[j,i]    -> matmul(lhsT=v tile,   rhs=exp tile)

V carries an appended ones column per head so row 64 of the O.T accumulator
is the softmax denominator; the numerator/denominator division happens on the
host after gather. Causal masking is a 0/1 multiply on the exp tile
(exp(-inf) == 0) over only the 128x128 triangle block where the diagonal
actually crosses; query columns that are fully below the diagonal are
column-skipped in the S matmul, the exp, and the O matmul. No max-subtraction
pass: scores*scale ~ N(0,1), so exp stays comfortably inside f32/bf16 range.

The attention phase is ScalarE(exp)-bound with ~25% PE idle, so the dt>=1
q/k projection matmul groups are drip-fed one per 5 attention steps to fill
the PE's exp-wait bubbles; head-pair p's attention emits the projections for
pair p+1, which complete before they are read. PSUM plan (8 banks): S
supertiles 2x2 (double-buffered), O accumulators 2x1 (single accumulator per
head; consecutive O matmuls alternate the two heads' banks so same-bank
accumulate turnaround is dodged), projection pool 2x1. Output blocks DMA
straight from PSUM to DRAM.
"""
import numpy as np

import bass_rust
import concourse.bass as bass
import concourse.mybir as mybir
import concourse.tile as tile
from concourse.bass_utils import run_bass_kernel_spmd

P = 128
HS = 64  # head size


def _split_sync_waits(nc, max_waits=1):
    # This walrus build's setupSyncWait admits a single sync-wait slot per
    # instruction, but Tile can emit several (cross-proc deps on one inst).
    # Peel extra waits onto preceding same-engine NOPs (pure wait carriers;
    # a Drain would flush the PE pipe).
    all_bbs = [b for fn in nc.m.functions for b in fn.blocks]
    for bb in all_bbs:
        insts = bb.instructions
        i = 0
        while i < len(insts):
            inst = insts[i]
            si = inst.sync_info
            ow = list(si.on_wait) if si and si.on_wait else []
            if len(ow) > max_waits:
                keep = ow[-max_waits:]
                rest = ow[:-max_waits]
                eng = nc.engines[inst.engine]
                new_insts = []
                while rest:
                    chunk, rest = rest[:max_waits], rest[max_waits:]
                    d = eng.nop()
                    d.ins.sync_info = bass_rust.SyncInfo(on_wait=chunk, on_update=[])
                    new_insts.append(d.ins)
                for bb2 in all_bbs:
                    ilist = bb2.instructions
                    changed = False
                    for ni in new_insts:
                        if ni in ilist:
                            ilist.remove(ni)
                            changed = True
                    if changed:
                        bb2.instructions = ilist
                si.on_wait = keep
                bb.instructions = insts[:i] + new_insts + insts[i:]
                insts = bb.instructions
                i += len(new_insts)
            i += 1
    return nc


def _make_tri():
    # tri[p, 0, i] = 1 if p <= i else 0 (the diagonal 128x128 causal triangle)
    import ml_dtypes

    p = np.arange(P)[:, None]
    i = np.arange(P)[None, :]
    return (p <= i).astype(ml_dtypes.bfloat16).reshape(P, 1, P)


def build_nc(T=2048, C=1024, D=512, FB=512, interleave=True):
    """One-core SPMD program: xT (C,T), wqT/wkT/wvT (C,D) bf16
    -> out_nd (H, HS+1, T) f32 numerators + denominator rows."""
    f32 = mybir.dt.float32
    bf16 = mybir.dt.bfloat16
    CK = C // P  # contraction subtiles
    DT = D // P  # q/k d-tiles
    TT = T // P  # t-tiles (v rows / key tiles)
    TB = T // FB  # query blocks
    JB = FB // P  # key tiles per query block
    H = D // HS  # local heads
    HPD = P // HS  # heads per d-tile (2)
    scale = float(HS) ** -0.5

    nc = bass.Bass()
    xT = nc.declare_dram_parameter("xT", [C, T], bf16, isOutput=False)
    wqT = nc.declare_dram_parameter("wqT", [C, D], bf16, isOutput=False)
    wkT = nc.declare_dram_parameter("wkT", [C, D], bf16, isOutput=False)
    wvT = nc.declare_dram_parameter("wvT", [C, D], bf16, isOutput=False)
    out_nd = nc.declare_dram_parameter("out_nd", [H, HS + 1, T], bf16, isOutput=True)
    tri = nc.inline_tensor(_make_tri(), name="tri_mask")

    xT_r = xT.rearrange("(ck p) t -> p ck t", p=P)
    wT_r = {w.name: w.rearrange("(ck p) d -> p ck d", p=P) for w in (wqT, wkT, wvT)}

    from contextlib import ExitStack

    with tile.TileContext(nc) as tc:
        with (
            tc.tile_pool(name="persist", bufs=1) as persist,
            tc.tile_pool(name="work", bufs=6) as work,
        ):
            # ---- load bf16 inputs (per-chunk tiles so deps stay fine-grained;
            # x/wq/wk chunks interleaved so q/k proj matmuls start as soon as
            # chunk 0 lands; wv after (v tiles are consumed later); tri last)
            # x chunks stream alone on the Sync HWDGE queue; the weights go on
            # the Scalar engine's HWDGE queue in parallel (ACT is idle during
            # the load), two chunks per descriptor to bound issue time
            x_bf = []
            w_bf = {w.name: [] for w in (wqT, wkT, wvT)}
            for ck in range(CK):
                t_ = persist.tile([P, T], bf16, name=f"x_bf{ck}", tag=f"x_bf{ck}")
                nc.sync.dma_start(t_[:], xT_r[:, ck, :])
                x_bf.append(t_)
            for w in (wqT, wkT, wvT):
                for ck2 in range(CK // 2):
                    t_ = persist.tile(
                        [P, 2, D], bf16, name=f"{w.name}_bf{ck2}", tag=f"{w.name}_bf{ck2}"
                    )
                    nc.scalar.dma_start(t_[:], wT_r[w.name][:, 2 * ck2 : 2 * ck2 + 2, :])
                    w_bf[w.name].append(t_)

            def w_sl(w, ck, cols):
                return w_bf[w.name][ck // 2][:, ck % 2, cols]

            tri_sb = persist.tile([P, 1, P], bf16, name="tri_sb", tag="tri_sb")
            nc.scalar.dma_start(tri_sb[:], tri[:])

            qT = [persist.tile([P, T], bf16, name=f"qT{dt}", tag=f"qT{dt}") for dt in range(DT)]
            kT = [persist.tile([P, T], bf16, name=f"kT{dt}", tag=f"kT{dt}") for dt in range(DT)]
            # v with an appended ones column per head: [P, TT, H*(HS+1)]
            v_sb = persist.tile([P, TT, H * (HS + 1)], bf16, tag="v_sb")

            def emit_qk_group(pool, w, dt, bi):
                # one projection group: CK accumulating matmuls + copy-out cast
                dst = qT[dt] if w is wqT else kT[dt]
                ps = pool.tile([P, FB], f32, name="ps_proj", tag="ps_proj")
                for ck in range(CK):
                    nc.tensor.matmul(
                        ps[:],
                        lhsT=w_sl(w, ck, slice(dt * P, (dt + 1) * P)),
                        rhs=x_bf[ck][:, bi * FB : (bi + 1) * FB],
                        start=(ck == 0),
                        stop=(ck == CK - 1),
                    )
                nc.vector.tensor_copy(dst[:, bi * FB : (bi + 1) * FB], ps[:])

            def emit_v_group(pool, tt):
                ps = pool.tile([P, D], f32, name="ps_proj", tag="ps_proj")
                for ck in range(CK):
                    nc.tensor.matmul(
                        ps[:],
                        lhsT=x_bf[ck][:, tt * P : (tt + 1) * P],
                        rhs=w_sl(wvT, ck, slice(0, D)),
                        start=(ck == 0),
                        stop=(ck == CK - 1),
                    )
                v3 = v_sb[:, tt, :].rearrange("p (h e) -> p h e", e=HS + 1)
                nc.vector.tensor_copy(v3[:, :, 0:HS], ps[:].rearrange("p (h e) -> p h e", e=HS))
                nc.vector.memset(v3[:, :, HS : HS + 1], 1.0)

            # ---- minimal upfront phase (wide PSUM ring so the chunk-paced
            # start keeps several accumulation groups in flight): q/k for
            # head-pair 0 / query-block 0, plus the first JB v tiles ----
            up_stack = ExitStack()
            ps_up = up_stack.enter_context(
                tc.tile_pool(name="ps_up", bufs=6, space="PSUM")
            )
            emit_qk_group(ps_up, wqT, 0, 0)
            emit_qk_group(ps_up, wkT, 0, 0)
            for tt in range(JB):
                emit_v_group(ps_up, tt)
            up_stack.close()

            # everything else drips into attention's PE bubbles as fillers,
            # ordered by its consumption deadline
            def filler_list():
                fills = []
                fills += [(emit_qk_group, wqT, 0, 1), (emit_qk_group, wkT, 0, 1)]
                fills += [(emit_v_group, tt) for tt in (4, 5)]
                fills += [(emit_qk_group, wqT, 0, 2), (emit_qk_group, wkT, 0, 2)]
                fills += [(emit_v_group, tt) for tt in (6, 7, 8, 9)]
                fills += [(emit_qk_group, wqT, 0, 3), (emit_qk_group, wkT, 0, 3)]
                fills += [(emit_v_group, tt) for tt in range(10, TT)]
                for dt in range(1, DT):
                    for w in (wqT, wkT):
                        for bi in range(TB):
                            fills.append((emit_qk_group, w, dt, bi))
                return fills

            fillers = iter(filler_list())

            def inject(pool, s):
                # deadline-driven: 2-of-3 steps early (head-pair-0 feeders +
                # dt1 due by step 40), then every 5th (dt2 by 80, dt3 by 120)
                if (s < 39 and s % 3 != 2) or (39 <= s < 120 and s % 5 == 0):
                    f = next(fillers, None)
                    if f is not None:
                        f[0](pool, *f[1:])

            if not interleave:
                for f in fillers:
                    f[0](ps_up, *f[1:])
                fillers = iter(())

            attn_stack = ExitStack()
            ps_s_pool = attn_stack.enter_context(
                tc.tile_pool(name="ps_s", bufs=2, space="PSUM")
            )
            ps_o_pool = attn_stack.enter_context(
                tc.tile_pool(name="ps_o", bufs=2, space="PSUM")
            )
            ps_proj = attn_stack.enter_context(
                tc.tile_pool(name="ps_proj", bufs=2, space="PSUM")
            )

            # ---- causal attention: one flat software-pipelined stream across
            # all (head-pair, query-block) blocks; S/exp of step s+1 issue
            # before O of step s, including across block boundaries, so the
            # next block's S-matmuls stream while the last O of the previous
            # block waits for its exp ----
            steps = [
                (hp, bi, jt)
                for hp in range(H // HPD)
                for bi in range(TB)
                for jt in range((bi + 1) * JB)
            ]
            ns = len(steps)
            LAG = 2  # O trails exp by 2 steps so O never waits on a fresh exp
            ps_o_map = {}
            ex_map = {}
            for s in range(ns + LAG):
                if s < ns:
                    hp, bi, jt = steps[s]
                    dt = hp
                    if jt == 0:
                        ps_o_map[(hp, bi)] = [
                            ps_o_pool.tile([HS + 1, FB], f32, name=f"ps_o{par}", tag="ps_o")
                            for par in range(HPD)
                        ]
                    l = jt - bi * JB  # >=0 on diagonal tiles
                    c0 = max(l, 0) * P  # fully-masked query-column prefix
                    # both heads' S tiles land in one 2-bank super-tile so a
                    # single EXP covers them (amortizes the ~200ns overhead)
                    sup = ps_s_pool.tile([P, HPD, FB], f32, name="ps_s", tag="ps_s")
                    for par in range(HPD):
                        po = HS * par
                        nc.tensor.matmul(
                            sup[:, par, c0:FB],
                            lhsT=kT[dt][po : po + HS, jt * P : (jt + 1) * P],
                            rhs=qT[dt][po : po + HS, bi * FB + c0 : (bi + 1) * FB],
                            start=True,
                            stop=True,
                        )
                    ex = work.tile([P, HPD, FB], bf16, name="exp", tag="exp")
                    nc.scalar.activation(
                        ex[:, :, c0:FB],
                        sup[:, :, c0:FB],
                        mybir.ActivationFunctionType.Exp,
                        scale=scale,
                    )
                    if l >= 0:
                        # only the 128-wide block where the diagonal crosses
                        # needs masking; columns beyond are fully visible,
                        # columns before were skipped
                        nc.vector.tensor_mul(
                            ex[:, :, c0 : c0 + P],
                            ex[:, :, c0 : c0 + P],
                            tri_sb[:].to_broadcast((P, HPD, P)),
                        )
                    ex_map[s] = (ex, c0)
                inject(ps_proj, s)
                if s >= LAG:
                    hp, bi, jt = steps[s - LAG]
                    nj = (bi + 1) * JB
                    ex, c0 = ex_map.pop(s - LAG)
                    ps_o = ps_o_map[(hp, bi)]
                    for par in range(HPD):
                        h = HPD * hp + par
                        nc.tensor.matmul(
                            ps_o[par][:, c0:FB],
                            lhsT=v_sb[:, jt, (HS + 1) * h : (HS + 1) * (h + 1)],
                            rhs=ex[:, par, c0:FB],
                            start=(jt == 0),
                            stop=(jt == nj - 1),
                        )
                    if jt == nj - 1:
                        for par in range(HPD):
                            h = HPD * hp + par
                            nd = work.tile([HS + 1, FB], bf16, name="nd", tag="nd")
                            nc.vector.tensor_copy(nd[:], ps_o[par][:])
                            nc.sync.dma_start(
                                out_nd[h, :, bi * FB : (bi + 1) * FB], nd[:]
                            )
                        del ps_o_map[(hp, bi)]

            for f in fillers:
                f[0](ps_proj, *f[1:])
            attn_stack.close()

    _split_sync_waits(nc)
    return nc


_NC_CACHE = {}


def _get_nc(key=(2048, 1024, 512, 512)):
    if key not in _NC_CACHE:
        _NC_CACHE[key] = build_nc(*key)
    return _NC_CACHE[key]


def run(x, Wq, Wk, Wv, trace=False, **spmd_kwargs):
    import ml_dtypes

    bf = ml_dtypes.bfloat16
    B, T, C = x.shape
    n_cores = 8
    gpb = 2  # head-groups per batch
    D = C // gpb
    H = D // HS

    nc = _get_nc((T, C, D, 512))

    in_maps = []
    for c in range(n_cores):
        b, g = c // gpb, c % gpb
        rows = slice(g * D, (g + 1) * D)
        in_maps.append(
            {
                "xT": np.ascontiguousarray(np.asarray(x)[b].T).astype(bf),
                "wqT": np.ascontiguousarray(np.asarray(Wq)[rows].T).astype(bf),
                "wkT": np.ascontiguousarray(np.asarray(Wk)[rows].T).astype(bf),
                "wvT": np.ascontiguousarray(np.asarray(Wv)[rows].T).astype(bf),
            }
        )

    res = run_bass_kernel_spmd(
        nc, in_maps, core_ids=list(range(n_cores)), trace=trace, **spmd_kwargs
    )

    out = np.empty((B, T, C), np.float32)
    for c in range(n_cores):
        b, g = c // gpb, c % gpb
        r = np.asarray(res.results[c]["out_nd"], np.float32)  # (H, HS+1, T)
        o = r[:, :HS, :] / r[:, HS : HS + 1, :]  # (H, HS, T)
        out[b, :, g * D : (g + 1) * D] = o.transpose(2, 0, 1).reshape(T, D)
    return out, res


def kernel(x, Wq, Wk, Wv, **_):
    out, _res = run(x, Wq, Wk, Wv, trace=False)
    return out


# revision 22
# speedup vs baseline: 1.0396x; 1.0233x over previous
"""Multi-head causal attention (B=4, T=2048, C=1024, H=16) on 8 TRN2 NeuronCores.

Sharding: core c handles batch b = c//2 and head-group g = c%2 (8 heads,
512 output channels). Host passes per-core transposed bf16 operands (x[b].T
and W[rows].T) so every on-device matmul streams natural-layout tiles:

  q.T[d,i] = sum_c WqT[c,d] * xT[c,i]   -> matmul(lhsT=WqT tile, rhs=xT tile)
  v[t,e]   = sum_c xT[c,t]  * WvT[c,e]  -> matmul(lhsT=xT tile,  rhs=WvT tile)
  S.T[j,i] = sum_d kT[d,j]  * qT[d,i]   -> matmul(lhsT=kT tile,  rhs=qT tile)
  O.T[e,i] = sum_j v[j,e]   * P[j,i]    -> matmul(lhsT=v tile,   rhs=exp tile)

V carries an appended ones column per head so row 64 of the O.T accumulator
is the softmax denominator; the numerator/denominator division happens on the
host after gather. Causal masking is a 0/1 multiply on the exp tile
(exp(-inf) == 0) over only the 128x128 triangle block where the diagonal
actually crosses; query columns that are fully below the diagonal are
column-skipped in the S matmul, the exp, and the O matmul. No max-subtraction
pass: scores*scale ~ N(0,1), so exp stays comfortably inside f32/bf16 range.

The attention phase is ScalarE(exp)-bound with ~25% PE idle, so the dt>=1
q/k projection matmul groups are drip-fed one per 5 attention steps to fill
the PE's exp-wait bubbles; head-pair p's attention emits the projections for
pair p+1, which complete before they are read. PSUM plan (8 banks): S
supertiles 2x2 (double-buffered), O accumulators 2x1 (single accumulator per
head; consecutive O matmuls alternate the two heads' banks so same-bank
accumulate turnaround is dodged), projection pool 2x1. Output blocks DMA
straight from PSUM to DRAM.
"""
import numpy as np

import bass_rust
import concourse.bass as bass
import concourse.mybir as mybir
import concourse.tile as tile
from concourse.bass_utils import run_bass_kernel_spmd

P = 128
HS = 64  # head size


def _split_sync_waits(nc, max_waits=1):
    # This walrus build's setupSyncWait admits a single sync-wait slot per
    # instruction, but Tile can emit several (cross-proc deps on one inst).
    # Peel extra waits onto preceding same-engine NOPs (pure wait carriers;
    # a Drain would flush the PE pipe).
    all_bbs = [b for fn in nc.m.functions for b in fn.blocks]
    for bb in all_bbs:
        insts = bb.instructions
        i = 0
        while i < len(insts):
            inst = insts[i]
            si = inst.sync_info
            ow = list(si.on_wait) if si and si.on_wait else []
            if len(ow) > max_waits:
                keep = ow[-max_waits:]
                rest = ow[:-max_waits]
                eng = nc.engines[inst.engine]
                new_insts = []
                while rest:
                    chunk, rest = rest[:max_waits], rest[max_waits:]
                    d = eng.nop()
                    d.ins.sync_info = bass_rust.SyncInfo(on_wait=chunk, on_update=[])
                    new_insts.append(d.ins)
                for bb2 in all_bbs:
                    ilist = bb2.instructions
                    changed = False
                    for ni in new_insts:
                        if ni in ilist:
                            ilist.remove(ni)
                            changed = True
                    if changed:
                        bb2.instructions = ilist
                si.on_wait = keep
                bb.instructions = insts[:i] + new_insts + insts[i:]
                insts = bb.instructions
                i += len(new_insts)
            i += 1
    return nc


def _make_tri():
    # tri[p, 0, i] = 1 if p <= i else 0 (the diagonal 128x128 causal triangle)
    import ml_dtypes

    p = np.arange(P)[:, None]
    i = np.arange(P)[None, :]
    return (p <= i).astype(ml_dtypes.bfloat16).reshape(P, 1, P)


def build_nc(T=2048, C=1024, D=512, FB=512, interleave=True):
    """One-core SPMD program: xT (C,T), wqT/wkT/wvT (C,D) bf16
    -> out_nd (H, HS+1, T) f32 numerators + denominator rows."""
    f32 = mybir.dt.float32
    bf16 = mybir.dt.bfloat16
    CK = C // P  # contraction subtiles
    DT = D // P  # q/k d-tiles
    TT = T // P  # t-tiles (v rows / key tiles)
    TB = T // FB  # query blocks
    JB = FB // P  # key tiles per query block
    H = D // HS  # local heads
    HPD = P // HS  # heads per d-tile (2)
    scale = float(HS) ** -0.5

    nc = bass.Bass()
    xT = nc.declare_dram_parameter("xT", [C, T], bf16, isOutput=False)
    wqT = nc.declare_dram_parameter("wqT", [C, D], bf16, isOutput=False)
    wkT = nc.declare_dram_parameter("wkT", [C, D], bf16, isOutput=False)
    wvT = nc.declare_dram_parameter("wvT", [C, D], bf16, isOutput=False)
    out_nd = nc.declare_dram_parameter("out_nd", [H, HS + 1, T], bf16, isOutput=True)
    tri = nc.inline_tensor(_make_tri(), name="tri_mask")

    xT_r = xT.rearrange("(ck p) t -> p ck t", p=P)
    wT_r = {w.name: w.rearrange("(ck p) d -> p ck d", p=P) for w in (wqT, wkT, wvT)}

    from contextlib import ExitStack

    with tile.TileContext(nc) as tc:
        with (
            tc.tile_pool(name="persist", bufs=1) as persist,
            tc.tile_pool(name="work", bufs=6) as work,
        ):
            # ---- load bf16 inputs (per-chunk tiles so deps stay fine-grained;
            # x/wq/wk chunks interleaved so q/k proj matmuls start as soon as
            # chunk 0 lands; wv after (v tiles are consumed later); tri last)
            # x chunks stream alone on the Sync HWDGE queue; the weights go on
            # the Scalar engine's HWDGE queue in parallel (ACT is idle during
            # the load), two chunks per descriptor to bound issue time
            x_bf = []
            w_bf = {w.name: [] for w in (wqT, wkT, wvT)}
            for ck in range(CK):
                t_ = persist.tile([P, T], bf16, name=f"x_bf{ck}", tag=f"x_bf{ck}")
                nc.sync.dma_start(t_[:], xT_r[:, ck, :])
                x_bf.append(t_)
            for w in (wqT, wkT, wvT):
                for ck2 in range(CK // 2):
                    t_ = persist.tile(
                        [P, 2, D], bf16, name=f"{w.name}_bf{ck2}", tag=f"{w.name}_bf{ck2}"
                    )
                    nc.scalar.dma_start(t_[:], wT_r[w.name][:, 2 * ck2 : 2 * ck2 + 2, :])
                    w_bf[w.name].append(t_)

            def w_sl(w, ck, cols):
                return w_bf[w.name][ck // 2][:, ck % 2, cols]

            tri_sb = persist.tile([P, 1, P], bf16, name="tri_sb", tag="tri_sb")
            nc.scalar.dma_start(tri_sb[:], tri[:])

            qT = [persist.tile([P, T], bf16, name=f"qT{dt}", tag=f"qT{dt}") for dt in range(DT)]
            kT = [persist.tile([P, T], bf16, name=f"kT{dt}", tag=f"kT{dt}") for dt in range(DT)]
            # v with an appended ones column per head: [P, TT, H*(HS+1)]
            v_sb = persist.tile([P, TT, H * (HS + 1)], bf16, tag="v_sb")

            def emit_qk_group(pool, w, dt, bi):
                # one projection group: CK accumulating matmuls + copy-out cast
                dst = qT[dt] if w is wqT else kT[dt]
                ps = pool.tile([P, FB], f32, name="ps_proj", tag="ps_proj")
                for ck in range(CK):
                    nc.tensor.matmul(
                        ps[:],
                        lhsT=w_sl(w, ck, slice(dt * P, (dt + 1) * P)),
                        rhs=x_bf[ck][:, bi * FB : (bi + 1) * FB],
                        start=(ck == 0),
                        stop=(ck == CK - 1),
                    )
                nc.vector.tensor_copy(dst[:, bi * FB : (bi + 1) * FB], ps[:])

            def emit_v_group(pool, tt):
                ps = pool.tile([P, D], f32, name="ps_proj", tag="ps_proj")
                for ck in range(CK):
                    nc.tensor.matmul(
                        ps[:],
                        lhsT=x_bf[ck][:, tt * P : (tt + 1) * P],
                        rhs=w_sl(wvT, ck, slice(0, D)),
                        start=(ck == 0),
                        stop=(ck == CK - 1),
                    )
                v3 = v_sb[:, tt, :].rearrange("p (h e) -> p h e", e=HS + 1)
                nc.vector.tensor_copy(v3[:, :, 0:HS], ps[:].rearrange("p (h e) -> p h e", e=HS))
                nc.vector.memset(v3[:, :, HS : HS + 1], 1.0)

            # ---- minimal upfront phase (wide PSUM ring so the chunk-paced
            # start keeps several accumulation groups in flight): q/k for
            # head-pair 0 / query-block 0, plus the first JB v tiles ----
            up_stack = ExitStack()
            ps_up = up_stack.enter_context(
                tc.tile_pool(name="ps_up", bufs=6, space="PSUM")
            )
            emit_qk_group(ps_up, wqT, 0, 0)
            emit_qk_group(ps_up, wkT, 0, 0)
            for tt in range(JB):
                emit_v_group(ps_up, tt)
            up_stack.close()

            # ---- causal attention runs blocks in bi-major order (all
            # head-pairs' query-block 0, then all query-block 1, ...) so each
            # projection group's first use — and hence its filler deadline —
            # spreads across the whole run instead of piling up by step 120 ----
            LAG = 2  # O trails exp by 2 steps so O never waits on a fresh exp
            LEAD = 10  # inject fillers this many steps before first use
            steps = [
                (hp, bi, jt)
                for bi in range(TB)
                for hp in range(H // HPD)
                for jt in range((bi + 1) * JB)
            ]
            ns = len(steps)
            block_start = {}
            for s, (hp, bi, jt) in enumerate(steps):
                if jt == 0:
                    block_start[(hp, bi)] = s

            # everything else drips into attention's PE bubbles as fillers,
            # injected just-in-time against its consumption deadline
            def filler_list():
                fills = []
                for dt in range(DT):
                    for bi in range(TB):
                        if dt == 0 and bi == 0:
                            continue  # upfront
                        dl = block_start[(dt, bi)]
                        fills.append((dl, (emit_qk_group, wqT, dt, bi)))
                        fills.append((dl + bi * JB, (emit_qk_group, wkT, dt, bi)))
                for tt in range(JB, TT):
                    t = tt // JB
                    dl = block_start[(0, t)] + (tt - t * JB) + LAG
                    fills.append((dl, (emit_v_group, tt)))
                fills.sort(key=lambda x: x[0])
                return fills

            fills = filler_list()
            fill_idx = [0]

            def inject(pool, s):
                n = 0
                while (
                    fill_idx[0] < len(fills)
                    and fills[fill_idx[0]][0] - LEAD <= s
                    and n < 2
                ):
                    f = fills[fill_idx[0]][1]
                    f[0](pool, *f[1:])
                    fill_idx[0] += 1
                    n += 1

            if not interleave:
                for _, f in fills:
                    f[0](ps_up, *f[1:])
                fill_idx[0] = len(fills)

            attn_stack = ExitStack()
            ps_s_pool = attn_stack.enter_context(
                tc.tile_pool(name="ps_s", bufs=2, space="PSUM")
            )
            ps_o_pool = attn_stack.enter_context(
                tc.tile_pool(name="ps_o", bufs=2, space="PSUM")
            )
            ps_proj = attn_stack.enter_context(
                tc.tile_pool(name="ps_proj", bufs=2, space="PSUM")
            )

            # ---- one flat software-pipelined stream across all blocks; S/exp
            # of step s issue before O of step s-LAG, including across block
            # boundaries, so the next block's S-matmuls stream while the last
            # O of the previous block waits for its exp ----
            ps_o_map = {}
            ex_map = {}
            for s in range(ns + LAG):
                if s < ns:
                    hp, bi, jt = steps[s]
                    dt = hp
                    if jt == 0:
                        ps_o_map[(hp, bi)] = [
                            ps_o_pool.tile([HS + 1, FB], f32, name=f"ps_o{par}", tag="ps_o")
                            for par in range(HPD)
                        ]
                    l = jt - bi * JB  # >=0 on diagonal tiles
                    c0 = max(l, 0) * P  # fully-masked query-column prefix
                    # both heads' S tiles land in one 2-bank super-tile so a
                    # single EXP covers them (amortizes the ~200ns overhead)
                    sup = ps_s_pool.tile([P, HPD, FB], f32, name="ps_s", tag="ps_s")
                    for par in range(HPD):
                        po = HS * par
                        nc.tensor.matmul(
                            sup[:, par, c0:FB],
                            lhsT=kT[dt][po : po + HS, jt * P : (jt + 1) * P],
                            rhs=qT[dt][po : po + HS, bi * FB + c0 : (bi + 1) * FB],
                            start=True,
                            stop=True,
                        )
                    ex = work.tile([P, HPD, FB], bf16, name="exp", tag="exp")
                    nc.scalar.activation(
                        ex[:, :, c0:FB],
                        sup[:, :, c0:FB],
                        mybir.ActivationFunctionType.Exp,
                        scale=scale,
                    )
                    if l >= 0:
                        # only the 128-wide block where the diagonal crosses
                        # needs masking; columns beyond are fully visible,
                        # columns before were skipped
                        nc.vector.tensor_mul(
                            ex[:, :, c0 : c0 + P],
                            ex[:, :, c0 : c0 + P],
                            tri_sb[:].to_broadcast((P, HPD, P)),
                        )
                    ex_map[s] = (ex, c0)
                inject(ps_proj, s)
                if s >= LAG:
                    hp, bi, jt = steps[s - LAG]
                    nj = (bi + 1) * JB
                    ex, c0 = ex_map.pop(s - LAG)
                    ps_o = ps_o_map[(hp, bi)]
                    for par in range(HPD):
                        h = HPD * hp + par
                        nc.tensor.matmul(
                            ps_o[par][:, c0:FB],
                            lhsT=v_sb[:, jt, (HS + 1) * h : (HS + 1) * (h + 1)],
                            rhs=ex[:, par, c0:FB],
                            start=(jt == 0),
                            stop=(jt == nj - 1),
                        )
                    if jt == nj - 1:
                        for par in range(HPD):
                            h = HPD * hp + par
                            nd = work.tile([HS + 1, FB], bf16, name="nd", tag="nd")
                            nc.vector.tensor_copy(nd[:], ps_o[par][:])
                            nc.sync.dma_start(
                                out_nd[h, :, bi * FB : (bi + 1) * FB], nd[:]
                            )
                        del ps_o_map[(hp, bi)]

            while fill_idx[0] < len(fills):
                f = fills[fill_idx[0]][1]
                f[0](ps_proj, *f[1:])
                fill_idx[0] += 1
            attn_stack.close()

    _split_sync_waits(nc)
    return nc


_NC_CACHE = {}


def _get_nc(key=(2048, 1024, 512, 512)):
    if key not in _NC_CACHE:
        _NC_CACHE[key] = build_nc(*key)
    return _NC_CACHE[key]


def run(x, Wq, Wk, Wv, trace=False, **spmd_kwargs):
    import ml_dtypes

    bf = ml_dtypes.bfloat16
    B, T, C = x.shape
    n_cores = 8
    gpb = 2  # head-groups per batch
    D = C // gpb
    H = D // HS

    nc = _get_nc((T, C, D, 512))

    in_maps = []
    for c in range(n_cores):
        b, g = c // gpb, c % gpb
        rows = slice(g * D, (g + 1) * D)
        in_maps.append(
            {
                "xT": np.ascontiguousarray(np.asarray(x)[b].T).astype(bf),
                "wqT": np.ascontiguousarray(np.asarray(Wq)[rows].T).astype(bf),
                "wkT": np.ascontiguousarray(np.asarray(Wk)[rows].T).astype(bf),
                "wvT": np.ascontiguousarray(np.asarray(Wv)[rows].T).astype(bf),
            }
        )

    res = run_bass_kernel_spmd(
        nc, in_maps, core_ids=list(range(n_cores)), trace=trace, **spmd_kwargs
    )

    out = np.empty((B, T, C), np.float32)
    for c in range(n_cores):
        b, g = c // gpb, c % gpb
        r = np.asarray(res.results[c]["out_nd"], np.float32)  # (H, HS+1, T)
        o = r[:, :HS, :] / r[:, HS : HS + 1, :]  # (H, HS, T)
        out[b, :, g * D : (g + 1) * D] = o.transpose(2, 0, 1).reshape(T, D)
    return out, res


def kernel(x, Wq, Wk, Wv, **_):
    out, _res = run(x, Wq, Wk, Wv, trace=False)
    return out
